# revision 1
# baseline (speedup 1.0000x reference)
"""Trainium2 Bass kernel for 4-directional Mamba with conv3d pre-stage.

Sharding: 8 cores = 4 scan directions x 2 batch elements. Each core runs the
full pipeline (pre-stage + one directional mamba) for its (dir, batch) pair;
direction flips are folded into host-side input prep:
  - channel flip  -> flip W_in columns / W_out rows
  - sequence flip -> feed spatially-flipped x + flipped depthwise conv taps
Host sums the 4 directions at the end.
"""
import sys

sys.path.insert(0, "/opt/trn_rl_repo/concourse")
sys.path.insert(0, "/opt/trn_rl_repo")

import numpy as np
import ml_dtypes

D_MODEL = 768
D_STATE = 64
D_CONV = 4
D_INNER = 1536
DT_RANK = 48
L = 2048
EPS = 1e-5
SLOPE = 0.01
G6 = 6      # d_model / 128
G12 = 12    # d_inner / 128
NT = 4      # 512-token chunks
CH = 512
GSZ = 8     # state-index group size
NGRP = D_STATE // GSZ
BF = np.float16

_CACHE = {}


def _taps():
    out = []
    for dd in (-1, 0, 1):
        for dh in (-1, 0, 1):
            for dw in (-1, 0, 1):
                out.append((dd, dh, dw))
    return out


def _build_program():
    import concourse.bass as bass
    import concourse.bacc as bacc
    import concourse.tile as tile
    from concourse import mybir

    f32 = mybir.dt.float32
    bf = mybir.dt.float16
    AF = mybir.ActivationFunctionType
    OP = mybir.AluOpType

    nc = bacc.Bacc()

    def din(name, shape, dt=f32):
        return nc.dram_tensor(name, shape, dt, kind="ExternalInput")

    x_in = din("x_in", [G6, 128, L], bf)
    bn_scale = din("bn_scale", [G6, 128, 1])
    bn_shift = din("bn_shift", [G6, 128, 1])
    dw_w = din("dw_w", [G6, 128, 27])
    pw_blk = din("pw_blk", [G6, G6, 128, 128], bf)        # [m][k]
    ln_g = din("ln_g", [G6, 128, 1])
    ln_b = din("ln_b", [G6, 128, 1])
    win_blk = din("win_blk", [2 * G12, G6, 128, 128], bf)  # [m][k]
    conv_w = din("conv_w", [G12, 128, D_CONV])
    conv_b = din("conv_b", [G12, 128, 1])
    w_xT = din("w_xT", [G12, 128, DT_RANK + 2 * D_STATE], bf)
    w_dtT = din("w_dtT", [DT_RANK, D_INNER], bf)
    b_dt = din("b_dt", [G12, 128, 1])
    a_cols = din("a_cols", [G12, 128, D_STATE])
    d_skip = din("d_skip", [G12, 128, 1])
    wout_blk = din("wout_blk", [G6, G12, 128, 128], bf)    # [m][k]
    ident_in = din("ident_in", [128, 128], bf)
    ones768 = din("ones768", [128, 1], bf)

    out_d = nc.dram_tensor("out", [G6, 128, L], f32, kind="ExternalOutput")

    TAPS = _taps()

    def bcast_row(src_row_ap, parts=128):
        # replicate a [1, N] DRAM row across `parts` partitions via DMA
        return bass.AP(tensor=src_row_ap.tensor, offset=src_row_ap.offset,
                       ap=[[0, parts]] + list(src_row_ap.ap[1:]))

    with tile.TileContext(nc) as tc:
        with (
            tc.tile_pool(name="wts", bufs=1) as wts,
            tc.tile_pool(name="wstream", bufs=24) as wstream,
            tc.tile_pool(name="fwork", bufs=2) as fwork,
            tc.tile_pool(name="bwork", bufs=2) as bwork,
            tc.tile_pool(name="stage", bufs=3) as stage,
            tc.tile_pool(name="mm", bufs=2, space="PSUM") as mm,
            tc.tile_pool(name="statps", bufs=2, space="PSUM") as statps,
            tc.tile_pool(name="ypsp", bufs=1, space="PSUM") as ypsp,
            tc.tile_pool(name="dram", bufs=1, space="DRAM") as dramp,
        ):
            # ---------- constants ----------
            def load1(name, src, shape, dt):
                t = wts.tile(shape, dt, tag=name, name=name)
                nc.sync.dma_start(out=t, in_=src)
                return t

            bnsc = [load1(f"bnsc{g}", bn_scale[g], [128, 1], f32) for g in range(G6)]
            bnsh = [load1(f"bnsh{g}", bn_shift[g], [128, 1], f32) for g in range(G6)]
            dww = [load1(f"dww{g}", dw_w[g], [128, 27], f32) for g in range(G6)]
            lng = [load1(f"lng{g}", ln_g[g], [128, 1], f32) for g in range(G6)]
            lnb = [load1(f"lnb{g}", ln_b[g], [128, 1], f32) for g in range(G6)]
            cvw = [load1(f"cvw{g}", conv_w[g], [128, D_CONV], f32) for g in range(G12)]
            cvb = [load1(f"cvb{g}", conv_b[g], [128, 1], f32) for g in range(G12)]
            bdt = [load1(f"bdt{g}", b_dt[g], [128, 1], f32) for g in range(G12)]
            acol = [load1(f"acol{g}", a_cols[g], [128, D_STATE], f32) for g in range(G12)]
            dsk = [load1(f"dsk{g}", d_skip[g], [128, 1], f32) for g in range(G12)]
            wdtT = load1("wdtT", w_dtT[:, :], [DT_RANK, D_INNER], bf)
            ident = load1("ident", ident_in[:, :], [128, 128], bf)
            o768 = load1("o768", ones768[:, :], [128, 1], bf)
            zcol = wts.tile([128, 1], f32, tag="zcol", name="zcol")
            nc.vector.memset(zcol, 0.0)
            epsc = wts.tile([1, 1], f32, tag="epsc", name="epsc")
            nc.vector.memset(epsc, EPS)

            # DRAM scratch
            z_sp = [dramp.tile([128, L], bf, tag=f"z_sp{g}", name=f"z_sp{g}")
                    for g in range(G12)]
            xma_sp = [dramp.tile([128, L], bf, tag=f"xma_sp{g}", name=f"xma_sp{g}")
                      for g in range(G12)]
            dt_sp = [dramp.tile([128, L], f32, tag=f"dt_sp{g}", name=f"dt_sp{g}")
                     for g in range(G12)]
            dtx_sp = [dramp.tile([128, L], bf, tag=f"dtx_sp{g}", name=f"dtx_sp{g}")
                      for g in range(G12)]
            bc_sp = dramp.tile([2 * D_STATE, L], bf, tag="bc_sp", name="bc_sp")
            mr_sp = dramp.tile([1, 2 * L], f32, tag="mr_sp", name="mr_sp")

            with tc.tile_pool(name="pxf", bufs=1) as pxf:
                xf = [pxf.tile([128, L], bf, tag=f"xf{g}", name=f"xf{g}")
                      for g in range(G6)]

                # ========== pre-stage ==========
                with tc.tile_pool(name="ppre", bufs=1) as ppre:
                    h1c = [ppre.tile([128, L], bf, tag=f"h1c{g}", name=f"h1c{g}")
                           for g in range(G6)]
                    for g in range(G6):
                        xp = ppre.tile([128, 10 * 18 * 18], bf, tag="xp", name="xp",
                                       bufs=2)
                        nc.gpsimd.memset(xp, 0.0)
                        xld = ppre.tile([128, L], bf, tag="xld", name="xld", bufs=2)
                        nc.sync.dma_start(out=xld, in_=x_in[g])
                        xp_v = xp.rearrange("p (d h w) -> p d h w", d=10, h=18, w=18)
                        xld_v = xld.rearrange("p (d h w) -> p d h w", d=8, h=16, w=16)
                        nc.scalar.activation(
                            xp_v[:, 1:9, 1:17, 1:17], xld_v,
                            AF.Prelu, bias=bnsh[g][:, 0:1], scale=bnsc[g][:, 0:1],
                            alpha=SLOPE)
                        diags = []
                        for ti in range(27):
                            dg = ppre.tile([128, 128], bf, tag="diag", name="diag",
                                           bufs=27)
                            nc.scalar.activation(dg, ident, AF.Copy, bias=0.0,
                                                 scale=dww[g][:, ti:ti + 1])
                            diags.append(dg)
                        for c in range(NT):
                            pc = mm.tile([128, CH], f32, tag="mmp", name="mmp")
                            for ti, (dd, dh, dw2) in enumerate(TAPS):
                                rhs = xp_v[:, 1 + dd + 2 * c: 3 + dd + 2 * c,
                                           1 + dh: 17 + dh, 1 + dw2: 17 + dw2]
                                nc.tensor.matmul(pc[:, :], diags[ti], rhs,
                                                 start=(ti == 0), stop=(ti == 26))
                            nc.scalar.copy(h1c[g][:, c * CH:(c + 1) * CH], pc[:, :])

                    # pointwise conv pass 1: stats only (h2 chunks discarded)
                    pw_all = []
                    for m in range(G6):
                        pw_m = []
                        for k in range(G6):
                            wt = ppre.tile([128, 128], bf, tag="pwall", name="pwall", bufs=36)
                            nc.sync.dma_start(out=wt, in_=pw_blk[m, k])
                            pw_m.append(wt)
                        pw_all.append(pw_m)
                    for c in range(NT):
                        mu_ps = statps.tile([1, CH], f32, tag="mups", name="mups", bufs=1)
                        var_ps = statps.tile([1, CH], f32, tag="vps", name="vps", bufs=1)
                        for m in range(G6):
                            pp = mm.tile([128, CH], f32, tag="mmp", name="mmp")
                            for k in range(G6):
                                nc.tensor.matmul(pp[:, :], pw_all[m][k],
                                                 h1c[k][:, c * CH:(c + 1) * CH],
                                                 start=(k == 0), stop=(k == G6 - 1))
                            ht = ppre.tile([128, CH], bf, tag="ht", name="ht", bufs=2)
                            nc.scalar.activation(ht, pp[:, :], AF.Prelu, bias=0.0,
                                                 scale=1.0, alpha=SLOPE)
                            nc.tensor.matmul(mu_ps[:, :], o768[:, 0:1], ht,
                                             start=(m == 0), stop=(m == G6 - 1))
                            sq = ppre.tile([128, CH], bf, tag="sq", name="sq", bufs=2)
                            nc.scalar.square(sq, ht)
                            nc.tensor.matmul(var_ps[:, :], o768[:, 0:1], sq,
                                             start=(m == 0), stop=(m == G6 - 1))
                        s1 = ppre.tile([1, CH], f32, tag="st1", name="st1", bufs=2)
                        nc.scalar.activation(s1, mu_ps[:, :], AF.Copy, bias=0.0,
                                             scale=1.0 / D_MODEL)
                        s2 = ppre.tile([1, CH], f32, tag="st2", name="st2", bufs=2)
                        nc.scalar.activation(s2, var_ps[:, :], AF.Copy, bias=0.0,
                                             scale=1.0 / D_MODEL)
                        s3 = ppre.tile([1, CH], f32, tag="st3", name="st3", bufs=2)
                        nc.scalar.square(s3, s1)
                        nc.vector.tensor_sub(s2, s2, s3)
                        nc.scalar.activation(s3, s2, AF.Sqrt,
                                             bias=epsc[0:1, 0:1], scale=1.0)
                        nc.vector.reciprocal(s3, s3)
                        nc.sync.dma_start(out=mr_sp[0:1, c * CH:(c + 1) * CH], in_=s1)
                        nc.sync.dma_start(out=mr_sp[0:1, L + c * CH:L + (c + 1) * CH],
                                          in_=s3)

                    murep = ppre.tile([128, L], bf, tag="murep", name="murep")
                    nc.gpsimd.dma_start(out=murep, in_=bcast_row(mr_sp[0:1, 0:L]))
                    rsrep = ppre.tile([128, L], bf, tag="rsrep", name="rsrep")
                    nc.gpsimd.dma_start(out=rsrep, in_=bcast_row(mr_sp[0:1, L:2 * L]))

                    # pass 2: recompute pw, apply leaky relu + layernorm -> xf
                    for m in range(G6):
                        for c in range(NT):
                            pp = mm.tile([128, CH], f32, tag="mmp", name="mmp")
                            for k in range(G6):
                                nc.tensor.matmul(pp[:, :], pw_all[m][k],
                                                 h1c[k][:, c * CH:(c + 1) * CH],
                                                 start=(k == 0), stop=(k == G6 - 1))
                            sl = c * CH
                            t1 = ppre.tile([128, CH], bf, tag="fc", name="fc", bufs=2)
                            nc.scalar.activation(t1, pp[:, :], AF.Prelu, bias=0.0,
                                                 scale=1.0, alpha=SLOPE)
                            nc.vector.tensor_sub(t1, t1, murep[:, sl:sl + CH])
                            nc.vector.tensor_mul(t1, t1, rsrep[:, sl:sl + CH])
                            nc.scalar.activation(xf[m][:, sl:sl + CH], t1, AF.Identity,
                                                 bias=lnb[m][:, 0:1],
                                                 scale=lng[m][:, 0:1])

                # ========== projections ==========
                with tc.tile_pool(name="pA", bufs=1) as pA:
                    xma = [pA.tile([128, L], bf, tag=f"xma{g}", name=f"xma{g}")
                           for g in range(G12)]
                    # in_proj xm blocks (m<12), fused with causal conv + silu
                    for m in range(G12):
                        win_m = []
                        for k in range(G6):
                            wt = wstream.tile([128, 128], bf, tag="wstr", name="wstr")
                            nc.sync.dma_start(out=wt, in_=win_blk[m, k])
                            win_m.append(wt)
                        xm_t = pA.tile([128, 3 + L], bf, tag="xm", name="xm_t", bufs=3)
                        nc.gpsimd.memset(xm_t[:, 0:3], 0.0)
                        for c in range(NT):
                            pp = mm.tile([128, CH], f32, tag="mmp", name="mmp")
                            for k in range(G6):
                                nc.tensor.matmul(pp[:, :], win_m[k],
                                                 xf[k][:, c * CH:(c + 1) * CH],
                                                 start=(k == 0), stop=(k == G6 - 1))
                            nc.scalar.copy(xm_t[:, 3 + c * CH: 3 + (c + 1) * CH],
                                           pp[:, :])
                        xc = pA.tile([128, L], bf, tag="xcs", name="xcs", bufs=2)
                        nc.scalar.activation(xc, xm_t[:, 0:L], AF.Copy, bias=0.0,
                                             scale=cvw[m][:, 0:1])
                        for j in range(1, D_CONV):
                            nc.vector.scalar_tensor_tensor(
                                xc, xm_t[:, j:j + L], cvw[m][:, j:j + 1], xc,
                                OP.mult, OP.add)
                        nc.scalar.activation(xma[m], xc, AF.Silu,
                                             bias=cvb[m][:, 0:1], scale=1.0)
                        nc.sync.dma_start(out=xma_sp[m], in_=xma[m])
                    # in_proj z blocks (m>=12) -> spill
                    for m in range(G12, 2 * G12):
                        win_m = []
                        for k in range(G6):
                            wt = wstream.tile([128, 128], bf, tag="wstr", name="wstr")
                            nc.sync.dma_start(out=wt, in_=win_blk[m, k])
                            win_m.append(wt)
                        for c in range(NT):
                            pp = mm.tile([128, CH], f32, tag="mmp", name="mmp")
                            for k in range(G6):
                                nc.tensor.matmul(pp[:, :], win_m[k],
                                                 xf[k][:, c * CH:(c + 1) * CH],
                                                 start=(k == 0), stop=(k == G6 - 1))
                            zst = pA.tile([128, CH], bf, tag="zst", name="zst", bufs=3)
                            nc.scalar.copy(zst, pp[:, :])
                            nc.sync.dma_start(
                                out=z_sp[m - G12][:, c * CH:(c + 1) * CH], in_=zst)

                    # x_proj -> dt_raw, B, C
                    wxT = [load1(f"wxT{g}", w_xT[g],
                                 [128, DT_RANK + 2 * D_STATE], bf)
                           for g in range(G12)]
                    dt_raw = pA.tile([DT_RANK, L], bf, tag="dtraw", name="dtraw")
                    bc_t = pA.tile([2 * D_STATE, L], bf, tag="bct", name="bct")
                    for dst, M, off in ((dt_raw, DT_RANK, 0),
                                        (bc_t[0:D_STATE, :], D_STATE, DT_RANK),
                                        (bc_t[D_STATE:2 * D_STATE, :], D_STATE,
                                         DT_RANK + D_STATE)):
                        for c in range(NT):
                            pp = mm.tile([128, CH], f32, tag="mmp", name="mmp")
                            for k in range(G12):
                                nc.tensor.matmul(pp[:M, :], wxT[k][:, off:off + M],
                                                 xma[k][:, c * CH:(c + 1) * CH],
                                                 start=(k == 0), stop=(k == G12 - 1))
                            nc.scalar.copy(dst[:, c * CH:(c + 1) * CH], pp[:M, :])
                    nc.sync.dma_start(out=bc_sp, in_=bc_t)

                    # dt = softplus(dt_proj + b_dt) via exp + log1p Taylor
                    for g in range(G12):
                        uf = fwork.tile([128, L], f32, tag="fa", name="fa")
                        for c in range(NT):
                            pp = mm.tile([128, CH], f32, tag="mmp", name="mmp")
                            nc.tensor.matmul(pp[:, :],
                                             wdtT[:, g * 128:(g + 1) * 128],
                                             dt_raw[:, c * CH:(c + 1) * CH],
                                             start=True, stop=True)
                            nc.scalar.activation(uf[:, c * CH:(c + 1) * CH],
                                                 pp[:, :], AF.Exp,
                                                 bias=bdt[g][:, 0:1], scale=1.0)
                        # 2-term log1p Taylor: u ~ 1e-2, truncation ~u^2/3
                        a = fwork.tile([128, L], f32, tag="fb", name="fb")
                        nc.vector.tensor_mul(a, uf, uf)
                        dtf = fwork.tile([128, L], f32, tag="fa", name="fa")
                        nc.vector.scalar_tensor_tensor(dtf, a, -0.5, uf,
                                                       OP.mult, OP.add)
                        dtx = bwork.tile([128, L], bf, tag="ba", name="ba")
                        nc.vector.tensor_mul(dtx, dtf, xma[g])
                        nc.sync.dma_start(out=dt_sp[g], in_=dtf)
                        nc.sync.dma_start(out=dtx_sp[g], in_=dtx)

            # ========== selective scan ==========
            with tc.tile_pool(name="pyall", bufs=1) as pyall:
                yall = [pyall.tile([128, L], bf, tag=f"yall{g}", name=f"yall{g}")
                        for g in range(G12)]
                with tc.tile_pool(name="pB", bufs=1) as pB:
                    for grp in range(NGRP):
                        breps, creps = [], []
                        for j in range(GSZ):
                            n = grp * GSZ + j
                            br = pB.tile([128, L], bf, tag="brep", name="brep",
                                         bufs=GSZ)
                            nc.gpsimd.dma_start(
                                out=br, in_=bcast_row(bc_sp[n:n + 1, :]))
                            cr = pB.tile([128, L], bf, tag="crep", name="crep",
                                         bufs=GSZ)
                            nc.gpsimd.dma_start(
                                out=cr, in_=bcast_row(
                                    bc_sp[D_STATE + n:D_STATE + n + 1, :]))
                            breps.append(br)
                            creps.append(cr)
                        for g in range(G12):
                            dt_db = fwork.tile([128, L], f32, tag="fa", name="fa")
                            nc.sync.dma_start(out=dt_db, in_=dt_sp[g])
                            dtx_db = bwork.tile([128, L], bf, tag="ba", name="ba")
                            nc.sync.dma_start(out=dtx_db, in_=dtx_sp[g])
                            yp = ypsp.tile([128, L], f32, tag="yps", name="yps")
                            if grp > 0:
                                # seed PSUM with the running sum (PE, not DVE)
                                for c in range(NT):
                                    nc.tensor.matmul(
                                        yp[:, c * CH:(c + 1) * CH], ident,
                                        yall[g][:, c * CH:(c + 1) * CH],
                                        start=True, stop=False)
                            for j in range(GSZ):
                                n = grp * GSZ + j
                                # fast-decaying states tolerate fp16 dA and get
                                # the DVE 2x mode on the scan
                                if n >= 16:
                                    dA = pB.tile([128, L], bf, tag="fbh",
                                                 name="fbh", bufs=2)
                                else:
                                    dA = fwork.tile([128, L], f32, tag="fb",
                                                    name="fb")
                                nc.scalar.activation(dA, dt_db, AF.Exp, bias=0.0,
                                                     scale=acol[g][:, n:n + 1])
                                u = bwork.tile([128, L], bf, tag="bu", name="bu")
                                nc.vector.tensor_mul(u, dtx_db, breps[j])
                                h = bwork.tile([128, L], bf, tag="bh", name="bh")
                                nc.vector.tensor_tensor_scan(h, dA, u, 0.0,
                                                             OP.mult, OP.add)
                                hc = bwork.tile([128, L], bf, tag="bhc", name="bhc")
                                nc.vector.tensor_mul(hc, h, creps[j])
                                for c in range(NT):
                                    nc.tensor.matmul(
                                        yp[:, c * CH:(c + 1) * CH], ident,
                                        hc[:, c * CH:(c + 1) * CH],
                                        start=(j == 0 and grp == 0),
                                        stop=(j == GSZ - 1))
                            nc.scalar.copy(yall[g], yp[:, :])

                # ========== gate + out_proj ==========
                with tc.tile_pool(name="pC", bufs=1) as pC:
                    yg = [pC.tile([128, L], bf, tag=f"yg{g}", name=f"yg{g}")
                          for g in range(G12)]
                    for g in range(G12):
                        z_db = bwork.tile([128, L], bf, tag="ba", name="ba")
                        nc.sync.dma_start(out=z_db, in_=z_sp[g])
                        xma_db = bwork.tile([128, L], bf, tag="bu", name="bu")
                        nc.sync.dma_start(out=xma_db, in_=xma_sp[g])
                        sz = bwork.tile([128, L], bf, tag="bh", name="bh")
                        nc.scalar.activation(sz, z_db, AF.Silu,
                                             bias=zcol[:, 0:1], scale=1.0)
                        t1 = pC.tile([128, L], bf, tag="gt1", name="gt1", bufs=2)
                        nc.vector.scalar_tensor_tensor(t1, xma_db, dsk[g][:, 0:1],
                                                       yall[g], OP.mult, OP.add)
                        nc.vector.tensor_mul(yg[g], t1, sz)
                    for m in range(G6):
                        wo_m = []
                        for k in range(G12):
                            wt = pC.tile([128, 128], bf, tag="wstr2",
                                         name="wstr2", bufs=24)
                            nc.sync.dma_start(out=wt, in_=wout_blk[m, k])
                            wo_m.append(wt)
                        for c in range(NT):
                            pp = mm.tile([128, CH], f32, tag="mmp", name="mmp")
                            for k in range(G12):
                                nc.tensor.matmul(pp[:, :], wo_m[k],
                                                 yg[k][:, c * CH:(c + 1) * CH],
                                                 start=(k == 0),
                                                 stop=(k == G12 - 1))
                            ob = pC.tile([128, CH], f32, tag="ob", name="ob", bufs=3)
                            nc.scalar.copy(ob, pp[:, :])
                            nc.sync.dma_start(out=out_d[m, :, c * CH:(c + 1) * CH],
                                              in_=ob)

    nc.compile()
    return nc


def _prep_core_inputs(inputs, dir_i, b):
    rev = dir_i >= 2
    cflip = (dir_i % 2) == 1
    f32 = np.float32

    xb = np.asarray(inputs["x"], f32)[b]
    if rev:
        xb = xb[:, ::-1, ::-1, ::-1]
    x_flat = np.ascontiguousarray(xb).reshape(G6, 128, L)

    bn_scale = (np.asarray(inputs["bn_gamma"], f32)
                / np.sqrt(np.asarray(inputs["bn_var"], f32) + EPS))
    bn_shift = (np.asarray(inputs["bn_beta"], f32)
                - np.asarray(inputs["bn_mean"], f32) * bn_scale)

    dww = np.asarray(inputs["dw_w"], f32)[:, 0]
    if rev:
        dww = dww[:, ::-1, ::-1, ::-1]
    dw_taps = np.ascontiguousarray(dww).reshape(D_MODEL, 27)

    W_in = np.asarray(inputs["W_in"], f32)
    if cflip:
        W_in = W_in[:, ::-1]
    W_out = np.asarray(inputs["W_out"], f32)
    if cflip:
        W_out = W_out[::-1, :]

    # blocked lhsT layouts: blk[m, k] = W.T[k*128:(k+1)*128, m*128:(m+1)*128]
    def blk(wT, km, mm_):
        # wT: [K, M] -> [M/128, K/128, 128, 128]
        K, M = wT.shape
        return np.ascontiguousarray(
            wT.reshape(km, 128, mm_, 128).transpose(2, 0, 1, 3))

    win_T = np.ascontiguousarray(W_in.T)        # [768, 3072]
    pw_T = np.ascontiguousarray(np.asarray(inputs["pw_w"], f32).T)  # [768,768]
    wout_T = np.ascontiguousarray(W_out.T)      # [1536, 768]

    a_neg = -np.exp(np.asarray(inputs["A_log"], f32))

    return {
        "x_in": x_flat.astype(BF),
        "bn_scale": bn_scale.reshape(G6, 128, 1),
        "bn_shift": bn_shift.reshape(G6, 128, 1),
        "dw_w": dw_taps.reshape(G6, 128, 27),
        "pw_blk": blk(pw_T, G6, G6).astype(BF),
        "ln_g": np.asarray(inputs["ln_gamma"], f32).reshape(G6, 128, 1),
        "ln_b": np.asarray(inputs["ln_beta"], f32).reshape(G6, 128, 1),
        "win_blk": blk(win_T, G6, 2 * G12).astype(BF),
        "conv_w": np.asarray(inputs["conv_w"], f32).reshape(G12, 128, D_CONV),
        "conv_b": np.asarray(inputs["conv_b"], f32).reshape(G12, 128, 1),
        "w_xT": np.ascontiguousarray(
            np.asarray(inputs["W_x"], f32).T).reshape(
                G12, 128, DT_RANK + 2 * D_STATE).astype(BF),
        "w_dtT": np.ascontiguousarray(np.asarray(inputs["W_dt"], f32).T).astype(BF),
        "b_dt": np.asarray(inputs["b_dt"], f32).reshape(G12, 128, 1),
        "a_cols": a_neg.reshape(G12, 128, D_STATE),
        "d_skip": np.asarray(inputs["D_skip"], f32).reshape(G12, 128, 1),
        "wout_blk": blk(wout_T, G12, G6).astype(BF),
        "ident_in": np.eye(128, dtype=f32).astype(BF),
        "ones768": np.ones((128, 1), f32).astype(BF),
    }


def kernel(**inputs):
    from concourse.bass_utils import run_bass_kernel_spmd

    if "nc" not in _CACHE:
        _CACHE["nc"] = _build_program()
    nc = _CACHE["nc"]

    in_maps = []
    for core in range(8):
        dir_i, b = core // 2, core % 2
        in_maps.append(_prep_core_inputs(inputs, dir_i, b))

    res = run_bass_kernel_spmd(nc, in_maps, core_ids=list(range(8)))

    B = np.asarray(inputs["x"]).shape[0]
    y = np.zeros((B, L, D_MODEL), np.float32)
    for core in range(8):
        dir_i, b = core // 2, core % 2
        oc = res.results[core]["out"].reshape(D_MODEL, L).T  # [L, 768]
        if dir_i >= 2:
            oc = oc[::-1, :]
        y[b] += oc
    y /= 4.0
    return y

